# revision 1
# baseline (speedup 1.0000x reference)
"""Bass/Trainium2 kernel for nn_CoreAttention (NTK causal attention with
linear phi-correction), SPMD over 8 NeuronCores.

Math (per batch b, head h; q,k,v: [n, d]; Z=phi_kv[h]: [d,d]; kk=|phi_k[h]|: [d,1]):
    phi_q  = ELU(q / d**0.25) + 1
    S      = q @ k.T / sqrt(d)
    A      = exp(S) * causal            # max-shift invariant -> use m=0
    num    = A @ v + phi_q @ Z
    den    = A @ ones + phi_q @ kk
    ctx    = num / den

Sharding: batch*head pairs (32) split 4-per-core across 8 cores. No
cross-core communication.

On-chip layout per (pair):
    qT, kT  : [64(d), 2048(n)] fp16 (host-pretransposed)
    phiT    : [64, 2048] fp16, computed on-chip, pre-scaled by 2^-7
    vp      : [128(k%128), 16(ktile), 65] fp16  (V with ones column appended)
    za      : [64, 65] fp16 = [Z | kk] * 2^7
Scores S^T [k,q] accumulate in PSUM (2-bank groups of two 128-k tiles x
512 q columns), exp on ScalarE (PSUM->SBUF fp16, scale=1/8 folded in),
diagonal 128x128 sub-blocks masked by an upper-triangular mask on VectorE,
then num[q, 4x65] accumulates in a single PSUM bank per 512-q block via
matmuls with expS^T slices as stationary weights. Division on VectorE.
"""

import math

import numpy as np

import concourse.bacc as bacc
import concourse.mybir as mybir
from concourse.tile import TileContext

SEQ, BS, H, D = 2048, 2, 16, 64
N_CORES = 8
NPAIR = BS * H            # 32 (b,h) pairs
PPC = NPAIR // N_CORES    # 4 pairs per core
P = 128                   # partition tile
NKT = SEQ // P            # 16 k tiles per pair
QB = 512                  # q-block width (one PSUM bank of fp32)
NQB = SEQ // QB           # 4 q blocks
QT_PER_B = QB // P        # 4 q tiles per block
DA = D + 1                # v augmented with ones column

_C = 1.0 / (D ** 0.25)        # phi input scale
_PHI_SCALE = 2.0 ** -7        # keep phi*Z product in fp16 normal range
_LN2_7 = 7.0 * math.log(2.0)
_EXP_SCALE = 1.0 / math.sqrt(D)

# Set by test harness only; grading path uses defaults.
TRACE = False
LAST_RESULT = None

_cached_nc = None


def _build_module():
    f16 = mybir.dt.float16
    f32 = mybir.dt.float32
    Exp = mybir.ActivationFunctionType.Exp
    Alu = mybir.AluOpType

    nc = bacc.Bacc("TRN2", target_bir_lowering=False, debug=False)

    d_qt = nc.dram_tensor("qt", [PPC, D, SEQ], f16, kind="ExternalInput")
    d_kt = nc.dram_tensor("kt", [PPC, D, SEQ], f16, kind="ExternalInput")
    d_vp = nc.dram_tensor("vp", [PPC, P, NKT, DA], f16, kind="ExternalInput")
    d_za = nc.dram_tensor("za", [PPC, D, DA], f16, kind="ExternalInput")
    d_tril = nc.dram_tensor("tril", [P, P], f16, kind="ExternalInput")
    d_out = nc.dram_tensor("out", [PPC, SEQ, D], f32, kind="ExternalOutput")

    with TileContext(nc) as tc:
        with (
            tc.tile_pool(name="const", bufs=1) as constp,
            tc.tile_pool(name="pairbuf", bufs=3) as pairp,
            tc.tile_pool(name="exbuf", bufs=6) as exp_pool,
            tc.tile_pool(name="scps", bufs=3, space="PSUM") as scp,
            tc.tile_pool(name="numps", bufs=2, space="PSUM") as nump,
            tc.tile_pool(name="outbuf", bufs=4) as outp,
        ):
            tril_t = constp.tile([P, P], f16)
            nc.sync.dma_start(out=tril_t, in_=d_tril[:, :])
            bias_t = constp.tile([P, 1], f32)
            nc.vector.memset(bias_t, -_LN2_7)

            # PE clock warm-up: the HAM un-throttles (1.2 -> 2.4 GHz) only
            # after a fully-busy activity window, which the dependency-laced
            # main loop never produces from cold. A dense dep-free burst
            # here (while the first DMAs stream in) warms the array; the
            # main loop's micro-gaps are too short to re-throttle it.
            warm_in = constp.tile([P, QB], f16)
            nc.vector.memset(warm_in, 0.0)
            wsc = scp.tile([P, 2, QB], f32, tag="sc")
            for w in range(18):
                # varying lhsT matters: a fixed one gets its LDWEIGHTS
                # elided and the stream never un-throttles
                nc.tensor.matmul(
                    out=wsc[:, w % 2, :],
                    lhsT=warm_in[:, (w % 4) * P: (w % 4 + 1) * P],
                    rhs=warm_in,
                    start=True, stop=True,
                )

            # Per-pair persistent tiles / per-qb num tiles, filled lazily by
            # the software-pipelined step loop below.
            pair_tiles = {}
            num_tiles = {}

            def load_pair(pair):
                if pair in pair_tiles or pair >= PPC:
                    return
                qT = pairp.tile([D, SEQ], f16, tag="qT")
                kT = pairp.tile([D, SEQ], f16, tag="kT")
                vp = pairp.tile([P, NKT, DA], f16, tag="vp")
                za = pairp.tile([D, DA], f16, tag="za")
                # chunked so the first QK group can start before the whole
                # pair has landed
                for ch in range(NQB):
                    s = slice(ch * QB, (ch + 1) * QB)
                    nc.sync.dma_start(out=qT[:, s], in_=d_qt[pair, :, s])
                    nc.sync.dma_start(out=kT[:, s], in_=d_kt[pair, :, s])
                nc.sync.dma_start(out=vp, in_=d_vp[pair])
                nc.sync.dma_start(out=za, in_=d_za[pair])
                # phiT = (exp(min(y,0)) + max(y,0)) * 2^-7,  y = q * _C
                # computed per q-chunk so qb=0's phi matmuls only wait on
                # chunk 0 of the qT DMA, not the whole pair load
                phiT = pairp.tile([D, SEQ], f16, tag="phiT")
                for ch in range(NQB):
                    s = slice(ch * QB, (ch + 1) * QB)
                    t_neg = pairp.tile([D, QB], f16, tag="t_neg")
                    t_lin = pairp.tile([D, QB], f16, tag="t_lin")
                    nc.vector.tensor_scalar(
                        out=t_neg, in0=qT[:, s], scalar1=-_C, scalar2=0.0,
                        op0=Alu.mult, op1=Alu.max,
                    )
                    nc.scalar.activation(
                        out=phiT[:, s], in_=t_neg, func=Exp, scale=-1.0,
                        bias=bias_t[:D, :],
                    )
                    nc.vector.tensor_scalar(
                        out=t_lin, in0=qT[:, s], scalar1=_C * _PHI_SCALE,
                        scalar2=0.0, op0=Alu.mult, op1=Alu.max,
                    )
                    nc.vector.tensor_add(
                        out=phiT[:, s], in0=phiT[:, s], in1=t_lin)
                pair_tiles[pair] = (qT, kT, vp, za, phiT)

            def c0_of(qb, j, u_pair_tile):
                # causal column restriction within the q-block for k-tile j.
                # The diag01 group's exp stays full-width: it reads stale
                # PSUM under t1's restricted range, but those ex columns are
                # never consumed by any AV matmul (qt >= t).
                t = j - 4 * qb
                if t >= 1:
                    return t * P
                return 0

            def emit_qk(step):
                pair, qb, g = step
                if qb == 0 and g == 0:
                    load_pair(pair)
                qT, kT, vp, za, phiT = pair_tiles[pair]
                q0 = qb * QB
                sc = scp.tile([P, 2, QB], f32, tag="sc")
                for u in range(2):
                    j = 2 * g + u
                    c0 = c0_of(qb, j, None)
                    nc.tensor.matmul(
                        out=sc[:, u, c0:QB],
                        lhsT=kT[:, j * P: (j + 1) * P],
                        rhs=qT[:, q0 + c0: q0 + QB],
                        start=True, stop=True,
                    )
                return sc

            steps = [(pair, qb, g)
                     for pair in range(PPC)
                     for qb in range(NQB)
                     for g in range(2 * (qb + 1))]
            LOOKAHEAD = 2
            sc_tiles = {}
            for i in range(min(LOOKAHEAD, len(steps))):
                sc_tiles[i] = emit_qk(steps[i])

            for i, step in enumerate(steps):
                if i + LOOKAHEAD < len(steps):
                    sc_tiles[i + LOOKAHEAD] = emit_qk(steps[i + LOOKAHEAD])
                pair, qb, g = step
                if qb == 1 and g == 0:
                    load_pair(pair + 1)   # prefetch next pair early
                qT, kT, vp, za, phiT = pair_tiles[pair]
                q0 = qb * QB
                sc = sc_tiles.pop(i)
                n_groups = 2 * (qb + 1)

                # exp (ScalarE, PSUM->SBUF fp16) + causal masks (VectorE)
                ex = exp_pool.tile([P, 2, QB], f16, tag="ex")
                ts = [2 * g - 4 * qb, 2 * g + 1 - 4 * qb]
                if ts[1] >= 2:  # (t2,t3) group: restricted exps
                    for u in range(2):
                        c0 = ts[u] * P
                        nc.scalar.activation(
                            out=ex[:, u, c0:QB], in_=sc[:, u, c0:QB],
                            func=Exp, scale=_EXP_SCALE,
                        )
                else:
                    nc.scalar.activation(
                        out=ex[:, :, :], in_=sc[:, :, :],
                        func=Exp, scale=_EXP_SCALE,
                    )
                for u in range(2):
                    t = ts[u]
                    if t >= 0:
                        nc.vector.tensor_mul(
                            out=ex[:, u, t * P:(t + 1) * P],
                            in0=ex[:, u, t * P:(t + 1) * P],
                            in1=tril_t,
                        )

                if g == 0:
                    # open the num accumulation group: phi_q @ [Z|kk]
                    num_t = nump.tile([P, QT_PER_B, DA], f32, tag="num")
                    num_tiles[(pair, qb)] = num_t
                    for qt in range(QT_PER_B):
                        nc.tensor.matmul(
                            out=num_t[:, qt, :],
                            lhsT=phiT[:, q0 + qt * P: q0 + (qt + 1) * P],
                            rhs=za,
                            start=(qt == 0), stop=False,
                        )
                num_t = num_tiles[(pair, qb)]

                for u in range(2):
                    j = 2 * g + u
                    t = j - 4 * qb
                    for qt in range(max(0, t), QT_PER_B):
                        last = (g == n_groups - 1 and u == 1
                                and qt == QT_PER_B - 1)
                        nc.tensor.matmul(
                            out=num_t[:, qt, :],
                            lhsT=ex[:, u, qt * P: (qt + 1) * P],
                            rhs=vp[:, j, :],
                            start=False, stop=last,
                        )

                if g == n_groups - 1:
                    num_tiles.pop((pair, qb))
                    rec = outp.tile([P, QT_PER_B, 1], f32, tag="rec")
                    nc.vector.reciprocal(out=rec, in_=num_t[:, :, D:DA])
                    out_t = outp.tile([P, QT_PER_B, D], f32, tag="out_t")
                    for qt in range(QT_PER_B):
                        nc.vector.tensor_scalar_mul(
                            out=out_t[:, qt, :],
                            in0=num_t[:, qt, 0:D],
                            scalar1=rec[:, qt, :],
                        )
                    nc.sync.dma_start(
                        out=d_out[pair, q0: q0 + QB, :].rearrange(
                            "(qt p) c -> p qt c", p=P),
                        in_=out_t,
                    )

    nc.compile()
    return nc


def _prep_core_inputs(query_layer, key_layer, value_layer, phi_k, phi_kv):
    q = np.asarray(query_layer, dtype=np.float32)
    k = np.asarray(key_layer, dtype=np.float32)
    v = np.asarray(value_layer, dtype=np.float32)
    zk = np.abs(np.asarray(phi_k, dtype=np.float32))[0, :, :, 0]   # [H, D]
    zv = np.asarray(phi_kv, dtype=np.float32)[0]                   # [H, D, D]

    # [seq,bs,h,d] -> per-pair transposed [pair, d, seq]
    qT = np.ascontiguousarray(q.transpose(1, 2, 3, 0).reshape(NPAIR, D, SEQ))
    kT = np.ascontiguousarray(k.transpose(1, 2, 3, 0).reshape(NPAIR, D, SEQ))

    vn = v.transpose(1, 2, 0, 3).reshape(NPAIR, SEQ, D)            # [pair, n, d]
    v_aug = np.concatenate(
        [vn, np.ones((NPAIR, SEQ, 1), np.float32)], axis=2)        # [pair, n, 65]
    vp = np.ascontiguousarray(
        v_aug.reshape(NPAIR, NKT, P, DA).transpose(0, 2, 1, 3))    # [pair, p, j, 65]

    za_h = np.concatenate([zv, zk[:, :, None]], axis=2) / _PHI_SCALE  # [H, D, 65]
    za = za_h[np.arange(NPAIR) % H]                                # [pair, d, 65]

    tril = np.triu(np.ones((P, P), np.float32))                    # keep k<=q in S^T

    in_maps = []
    for c in range(N_CORES):
        s = slice(c * PPC, (c + 1) * PPC)
        in_maps.append({
            "qt": qT[s].astype(np.float16),
            "kt": kT[s].astype(np.float16),
            "vp": vp[s].astype(np.float16),
            "za": za[s].astype(np.float16),
            "tril": tril.astype(np.float16),
        })
    return in_maps


def _install_trace_shim():
    import sys
    import types
    if "antenv.axon_hooks" not in sys.modules:
        m = types.ModuleType("antenv.axon_hooks")
        m._hook = None
        m.set_axon_ntff_profile_hook = lambda h: setattr(m, "_hook", h)
        m.get_axon_ntff_profile_hook = lambda: m._hook
        sys.modules["antenv.axon_hooks"] = m
        import antenv
        antenv.axon_hooks = m
    from trn_agent_boot.trn_boot import _ntff_profile_via_ctypes
    sys.modules["antenv.axon_hooks"].set_axon_ntff_profile_hook(
        _ntff_profile_via_ctypes("/opt/axon/libaxon_pjrt.so"))
    import concourse.bass_utils as bu
    bu.upload_artifacts = lambda tmpdir: "local://" + str(tmpdir)


def kernel(query_layer, key_layer, value_layer, attention_mask, phi_k, phi_kv):
    global _cached_nc, LAST_RESULT
    from concourse.bass_utils import run_bass_kernel_spmd

    if TRACE:
        _install_trace_shim()
    if _cached_nc is None:
        _cached_nc = _build_module()
    nc = _cached_nc

    in_maps = _prep_core_inputs(
        query_layer, key_layer, value_layer, phi_k, phi_kv)
    res = run_bass_kernel_spmd(
        nc, in_maps, core_ids=list(range(N_CORES)), trace=TRACE)
    LAST_RESULT = res

    outs = np.stack([res.results[c]["out"] for c in range(N_CORES)])  # [8,4,n,d]
    ctx = outs.reshape(BS, H, SEQ, D).transpose(2, 0, 1, 3)           # [n,bs,h,d]
    return np.ascontiguousarray(ctx.reshape(SEQ, BS, H * D)).astype(np.float32)



# revision 2
# speedup vs baseline: 1.5120x; 1.5120x over previous
"""Bass/Trainium2 kernel for nn_CoreAttention (NTK causal attention with
linear phi-correction), SPMD over 8 NeuronCores.

Math (per batch b, head h; q,k,v: [n, d]; Z=phi_kv[h]: [d,d]; kk=|phi_k[h]|: [d,1]):
    phi_q  = ELU(q / d**0.25) + 1
    S      = q @ k.T / sqrt(d)
    A      = exp(S) * causal            # max-shift invariant -> use m=0
    num    = A @ v + phi_q @ Z
    den    = A @ ones + phi_q @ kk
    ctx    = num / den

Sharding: batch*head pairs (32) split 4-per-core across 8 cores. No
cross-core communication.

v2 design notes (vs the ScalarE-exp-serialized v1 baseline at 133.5us):
  * phi_q computed on HOST (it is an elementwise transform of the q input)
    and DMA'd in as phiT, removing the on-device ELU chain (ScalarE+DVE).
  * exp is split across TWO engines: ScalarE runs the exact Exp ACT for
    most non-diagonal score groups; VectorE runs a Schraudolph bit-trick
    exp (bits = round(S*A + B) as int16, reinterpreted as fp16) for the
    diagonal groups plus a share of the rest.  The causal mask is folded
    into the VectorE exp via a bias-mask tile: masked lanes get -1e6,
    which saturates the int16 convert to -32768 = 0x8000 = fp16 -0.0.
  * QK matmuls are row-tiled 2x (contraction is only d=64, so k-tile j
    goes to PE rows 0-63 and j+1 to rows 64-127 concurrently via
    tile_position); q/k operands are host-duplicated into both halves.
  * Output DRAM layout matches the on-chip tile exactly (no rearrange),
    host fixes the layout during the gather.

On-chip layout per (pair):
    qk      : [128, 2, 2048] fp16  (qT dup'd in both 64-row halves, kT same)
    phiT    : [64, 2048] fp16, host-computed, pre-scaled by 2^-7
    vp      : [128(k%128), 16(ktile), 65] fp16  (V with ones column appended)
    za      : [64, 65] fp16 = [Z | kk] * 2^7
Scores S^T [k,q] accumulate in PSUM (2-bank groups of two 128-k tiles x
512 q columns), exp'd to fp16/int16 SBUF as above, then num[q, 4x65]
accumulates in a single PSUM bank per 512-q block via matmuls with expS^T
slices as stationary weights.  Division on VectorE.
"""

import math

import numpy as np

import concourse.bacc as bacc
import concourse.mybir as mybir
from concourse.tile import TileContext

SEQ, BS, H, D = 2048, 2, 16, 64
N_CORES = 8
NPAIR = BS * H            # 32 (b,h) pairs
PPC = NPAIR // N_CORES    # 4 pairs per core
P = 128                   # partition tile
NKT = SEQ // P            # 16 k tiles per pair
QB = 512                  # q-block width (one PSUM bank of fp32)
NQB = SEQ // QB           # 4 q blocks
QT_PER_B = QB // P        # 4 q tiles per block
DA = D + 1                # v augmented with ones column

_C = 1.0 / (D ** 0.25)        # phi input scale
_PHI_SCALE = 2.0 ** -7        # keep phi*Z product in fp16 normal range
_EXP_SCALE = 1.0 / math.sqrt(D)

# Schraudolph fp16 exp: bits = round(S * A + B); bits.view(fp16) ~ exp(S/8).
# A = 1024 * log2(e) / 8;  B = 15*1024 - 60 (offset tuned for min ctx L2).
_SCH_A = 1024.0 * math.log2(math.e) / 8.0
_SCH_B = 15.0 * 1024.0 - 60.0
_SCH_MASKED = -1.0e6          # saturates int16 convert -> 0x8000 -> fp16 -0.0

# Set by test harness only; grading path uses defaults.
TRACE = False
LAST_RESULT = None

_cached_nc = None


def _exp_engine(qb, g):
    """'ve' = VectorE Schraudolph, 'sc' = ScalarE exact Exp."""
    if g >= 2 * qb:           # the two diagonal-containing groups of each qb
        return 've'
    if (qb, g) in ((2, 0), (3, 0)):   # load balance
        return 've'
    return 'sc'


def _build_module():
    f16 = mybir.dt.float16
    f32 = mybir.dt.float32
    i16 = mybir.dt.int16
    Exp = mybir.ActivationFunctionType.Exp
    Alu = mybir.AluOpType

    nc = bacc.Bacc("TRN2", target_bir_lowering=False, debug=False)

    d_qk = nc.dram_tensor("qk", [PPC, P, 2, SEQ], f16, kind="ExternalInput")
    d_ph = nc.dram_tensor("ph", [PPC, D, SEQ], f16, kind="ExternalInput")
    d_vp = nc.dram_tensor("vp", [PPC, P, NKT, DA], f16, kind="ExternalInput")
    d_za = nc.dram_tensor("za", [PPC, D, DA], f16, kind="ExternalInput")
    d_bm = nc.dram_tensor("bm", [P, 2, QB], f32, kind="ExternalInput")
    d_out = nc.dram_tensor("out", [PPC, NQB, P, QT_PER_B, D], f32,
                           kind="ExternalOutput")

    with TileContext(nc) as tc:
        with (
            tc.tile_pool(name="const", bufs=1) as constp,
            tc.tile_pool(name="pairbuf", bufs=3) as pairp,
            tc.tile_pool(name="exbuf", bufs=6) as exp_pool,
            tc.tile_pool(name="scps", bufs=3, space="PSUM") as scp,
            tc.tile_pool(name="numps", bufs=2, space="PSUM") as nump,
            tc.tile_pool(name="outbuf", bufs=4) as outp,
        ):
            bm_t = constp.tile([P, 2, QB], f32)
            nc.sync.dma_start(out=bm_t, in_=d_bm[:, :, :])

            # PE clock warm-up: the HAM un-throttles (1.2 -> 2.4 GHz) only
            # after a fully-busy activity window, which a dependency-laced
            # loop never produces from cold.  A dense dep-free burst here
            # (while the first DMAs stream in) warms the array.
            warm_in = constp.tile([P, QB], f16)
            nc.vector.memset(warm_in, 0.0)
            wsc = scp.tile([P, 2, QB], f32, tag="sc")
            for w in range(18):
                # varying lhsT matters: a fixed one gets its LDWEIGHTS
                # elided and the stream never un-throttles
                nc.tensor.matmul(
                    out=wsc[:, w % 2, :],
                    lhsT=warm_in[:, (w % 4) * P: (w % 4 + 1) * P],
                    rhs=warm_in,
                    start=True, stop=True,
                )

            pair_tiles = {}
            num_tiles = {}

            def load_pair(pair):
                if pair in pair_tiles or pair >= PPC:
                    return
                qk = pairp.tile([P, 2, SEQ], f16, tag="qk")
                phiT = pairp.tile([D, SEQ], f16, tag="phiT")
                vp = pairp.tile([P, NKT, DA], f16, tag="vp")
                za = pairp.tile([D, DA], f16, tag="za")
                if pair == 0:
                    # chunked so the first QK group can start before the
                    # whole pair has landed
                    nc.sync.dma_start(out=qk[:, :, 0:QB],
                                      in_=d_qk[pair, :, :, 0:QB])
                    nc.sync.dma_start(out=qk[:, :, QB:SEQ],
                                      in_=d_qk[pair, :, :, QB:SEQ])
                else:
                    nc.sync.dma_start(out=qk, in_=d_qk[pair])
                nc.sync.dma_start(out=phiT, in_=d_ph[pair])
                nc.sync.dma_start(out=vp, in_=d_vp[pair])
                nc.sync.dma_start(out=za, in_=d_za[pair])
                pair_tiles[pair] = (qk, phiT, vp, za)

            def c0_of(qb, j):
                # causal column restriction within the q-block for k-tile j
                t = j - 4 * qb
                if t >= 1:
                    return t * P
                return 0

            def emit_qk(step):
                pair, qb, g = step
                if qb == 0 and g == 0:
                    load_pair(pair)
                qk, phiT, vp, za = pair_tiles[pair]
                q0 = qb * QB
                sc = scp.tile([P, 2, QB], f32, tag="sc")
                for u in range(2):
                    j = 2 * g + u
                    c0 = c0_of(qb, j)
                    h = 64 * u
                    nc.tensor.matmul(
                        out=sc[:, u, c0:QB],
                        lhsT=qk[h:h + 64, 1, j * P: (j + 1) * P],
                        rhs=qk[h:h + 64, 0, q0 + c0: q0 + QB],
                        start=True, stop=True,
                        tile_position=(h, 0),
                    )
                return sc

            def emit_exp(step, sc):
                """exp the score group; returns (ex_tile, is_i16)."""
                pair, qb, g = step
                if _exp_engine(qb, g) == 'sc':
                    ex = exp_pool.tile([P, 2, QB], f16, tag="exf")
                    nc.scalar.activation(
                        out=ex[:, :, :], in_=sc[:, :, :],
                        func=Exp, scale=_EXP_SCALE,
                    )
                    return ex, False
                ex = exp_pool.tile([P, 2, QB], i16, tag="exi")
                if g < 2 * qb:
                    # plain group: bits = sc*A + B
                    nc.vector.tensor_scalar(
                        out=ex[:, :, :], in0=sc[:, :, :],
                        scalar1=_SCH_A, scalar2=_SCH_B,
                        op0=Alu.mult, op1=Alu.add,
                    )
                elif g == 2 * qb:
                    # (t0,t1) group: full width, mask folded via bm_t
                    nc.vector.scalar_tensor_tensor(
                        out=ex[:, :, :], in0=sc[:, :, :],
                        scalar=_SCH_A, in1=bm_t[:, :, :],
                        op0=Alu.mult, op1=Alu.add,
                    )
                else:
                    # (t2,t3) group: only the causally-valid column ranges.
                    # bm_t[:, 0, 0:128] is the triangular pattern and
                    # bm_t[:, 0, 128:256] is all-B, so slices of bm_t line
                    # up with both regions.
                    nc.vector.scalar_tensor_tensor(
                        out=ex[:, 0, 2 * P:QB], in0=sc[:, 0, 2 * P:QB],
                        scalar=_SCH_A, in1=bm_t[:, 0, 0:2 * P],
                        op0=Alu.mult, op1=Alu.add,
                    )
                    nc.vector.scalar_tensor_tensor(
                        out=ex[:, 1, 3 * P:QB], in0=sc[:, 1, 3 * P:QB],
                        scalar=_SCH_A, in1=bm_t[:, 0, 0:P],
                        op0=Alu.mult, op1=Alu.add,
                    )
                return ex, True

            steps = [(pair, qb, g)
                     for pair in range(PPC)
                     for qb in range(NQB)
                     for g in range(2 * (qb + 1))]
            LOOKAHEAD = 2
            sc_tiles = {}
            for i in range(min(LOOKAHEAD, len(steps))):
                sc_tiles[i] = emit_qk(steps[i])

            for i, step in enumerate(steps):
                if i + LOOKAHEAD < len(steps):
                    sc_tiles[i + LOOKAHEAD] = emit_qk(steps[i + LOOKAHEAD])
                pair, qb, g = step
                if qb == 1 and g == 0:
                    load_pair(pair + 1)   # prefetch next pair early
                qk, phiT, vp, za = pair_tiles[pair]
                q0 = qb * QB
                sc = sc_tiles.pop(i)
                n_groups = 2 * (qb + 1)

                ex, is_i16 = emit_exp(step, sc)

                if g == 0:
                    # open the num accumulation group: phi_q @ [Z|kk]
                    num_t = nump.tile([P, QT_PER_B, DA], f32, tag="num")
                    num_tiles[(pair, qb)] = num_t
                    for qt in range(QT_PER_B):
                        nc.tensor.matmul(
                            out=num_t[:, qt, :],
                            lhsT=phiT[:, q0 + qt * P: q0 + (qt + 1) * P],
                            rhs=za,
                            start=(qt == 0), stop=False,
                        )
                num_t = num_tiles[(pair, qb)]

                for u in range(2):
                    j = 2 * g + u
                    t = j - 4 * qb
                    for qt in range(max(0, t), QT_PER_B):
                        last = (g == n_groups - 1 and u == 1
                                and qt == QT_PER_B - 1)
                        lhsT = ex[:, u, qt * P: (qt + 1) * P]
                        if is_i16:
                            lhsT = lhsT.bitcast(f16)
                        nc.tensor.matmul(
                            out=num_t[:, qt, :],
                            lhsT=lhsT,
                            rhs=vp[:, j, :],
                            start=False, stop=last,
                        )

                if g == n_groups - 1:
                    num_tiles.pop((pair, qb))
                    rec = outp.tile([P, QT_PER_B, 1], f32, tag="rec")
                    nc.vector.reciprocal(out=rec, in_=num_t[:, :, D:DA])
                    out_t = outp.tile([P, QT_PER_B, D], f32, tag="out_t")
                    for qt in range(QT_PER_B):
                        nc.vector.tensor_scalar_mul(
                            out=out_t[:, qt, :],
                            in0=num_t[:, qt, 0:D],
                            scalar1=rec[:, qt, :],
                        )
                    nc.sync.dma_start(out=d_out[pair, qb], in_=out_t)

    nc.compile()
    return nc


def _prep_core_inputs(query_layer, key_layer, value_layer, phi_k, phi_kv):
    q = np.asarray(query_layer, dtype=np.float32)
    k = np.asarray(key_layer, dtype=np.float32)
    v = np.asarray(value_layer, dtype=np.float32)
    zk = np.abs(np.asarray(phi_k, dtype=np.float32))[0, :, :, 0]   # [H, D]
    zv = np.asarray(phi_kv, dtype=np.float32)[0]                   # [H, D, D]

    # [seq,bs,h,d] -> per-pair transposed [pair, d, seq]
    qT = np.ascontiguousarray(q.transpose(1, 2, 3, 0).reshape(NPAIR, D, SEQ))
    kT = np.ascontiguousarray(k.transpose(1, 2, 3, 0).reshape(NPAIR, D, SEQ))

    # interleave q/k and duplicate into both 64-row halves for PE row tiling
    qkt = np.stack([qT, kT], axis=2)                # [pair, 64, 2, seq]
    qk2 = np.concatenate([qkt, qkt], axis=1)        # [pair, 128, 2, seq]

    # host phi: ELU(q*_C) + 1 = (q*_C + 1) if >0-branch else exp(q*_C)
    xs = qT * _C
    ph = np.where(xs > 0.0, xs + 1.0, np.exp(np.minimum(xs, 0.0)))
    ph = ph * _PHI_SCALE                            # [pair, 64, seq]

    vn = v.transpose(1, 2, 0, 3).reshape(NPAIR, SEQ, D)            # [pair, n, d]
    v_aug = np.concatenate(
        [vn, np.ones((NPAIR, SEQ, 1), np.float32)], axis=2)        # [pair, n, 65]
    vp = np.ascontiguousarray(
        v_aug.reshape(NPAIR, NKT, P, DA).transpose(0, 2, 1, 3))    # [pair, p, j, 65]

    za_h = np.concatenate([zv, zk[:, :, None]], axis=2) / _PHI_SCALE  # [H, D, 65]
    za = za_h[np.arange(NPAIR) % H]                                # [pair, d, 65]

    # bias-mask tile for the VectorE Schraudolph exp.
    # [p, 0, 0:128]   : triangular (valid iff q >= k within the diag block)
    # [p, 0, 128:512] : all valid
    # [p, 1, 0:128]   : all masked (those columns are never read)
    # [p, 1, 128:256] : triangular; [p, 1, 256:512]: valid
    tri = np.where(np.arange(P)[None, :] >= np.arange(P)[:, None],
                   _SCH_B, _SCH_MASKED).astype(np.float32)         # [k, q]
    bm = np.full((P, 2, QB), _SCH_B, np.float32)
    bm[:, 0, 0:P] = tri
    bm[:, 1, 0:P] = _SCH_MASKED
    bm[:, 1, P:2 * P] = tri

    in_maps = []
    for c in range(N_CORES):
        s = slice(c * PPC, (c + 1) * PPC)
        in_maps.append({
            "qk": qk2[s].astype(np.float16),
            "ph": ph[s].astype(np.float16),
            "vp": vp[s].astype(np.float16),
            "za": za[s].astype(np.float16),
            "bm": bm,
        })
    return in_maps


def _install_trace_shim():
    import sys
    import types
    if "antenv.axon_hooks" not in sys.modules:
        m = types.ModuleType("antenv.axon_hooks")
        m._hook = None
        m.set_axon_ntff_profile_hook = lambda h: setattr(m, "_hook", h)
        m.get_axon_ntff_profile_hook = lambda: m._hook
        sys.modules["antenv.axon_hooks"] = m
        import antenv
        antenv.axon_hooks = m
    from trn_agent_boot.trn_boot import _ntff_profile_via_ctypes
    sys.modules["antenv.axon_hooks"].set_axon_ntff_profile_hook(
        _ntff_profile_via_ctypes("/opt/axon/libaxon_pjrt.so"))
    import concourse.bass_utils as bu
    bu.upload_artifacts = lambda tmpdir: "local://" + str(tmpdir)


def kernel(query_layer, key_layer, value_layer, attention_mask, phi_k, phi_kv):
    global _cached_nc, LAST_RESULT
    from concourse.bass_utils import run_bass_kernel_spmd

    if TRACE:
        _install_trace_shim()
    if _cached_nc is None:
        _cached_nc = _build_module()
    nc = _cached_nc

    in_maps = _prep_core_inputs(
        query_layer, key_layer, value_layer, phi_k, phi_kv)
    res = run_bass_kernel_spmd(
        nc, in_maps, core_ids=list(range(N_CORES)), trace=TRACE)
    LAST_RESULT = res

    outs = np.stack([res.results[c]["out"] for c in range(N_CORES)])
    # [8, PPC, NQB, P, QT, D] -> row q = qb*512 + qt*128 + p
    outs = outs.reshape(NPAIR, NQB, P, QT_PER_B, D)
    ctx = outs.transpose(0, 1, 3, 2, 4).reshape(BS, H, SEQ, D)
    ctx = ctx.transpose(2, 0, 1, 3)                               # [n,bs,h,d]
    return np.ascontiguousarray(ctx.reshape(SEQ, BS, H * D)).astype(np.float32)
